# revision 10
# baseline (speedup 1.0000x reference)
"""Trainium2 Bass kernel for nn_CubeSimulator.

Reference computation: a 128^3 spatial grid is rotated (Rz(sky_rot) then
Rx(inclination)), a rotation-curve velocity field and an exponential-disk
intensity field are evaluated, an 80-channel Gaussian KDE over the
line-of-sight velocity reduces the third grid axis, and the [80,128,128]
cube is avg-pooled (5,4,4) to [16,32,32].

Kernel strategy
---------------
* Point-reflection symmetry: cube[v,127-i,127-j] == cube[79-v,i,j], so only
  the i < 64 half-grid runs on-device; the host mirrors the pooled output.
* Sharding: 8 sky-plane rows per core over 8 NeuronCores (data-parallel over
  image rows).  On-device layout: partitions = k (the reduced axis), free =
  (i_local=8) x (j=128) = 1024.
* Geometric-chain KDE (the main trick): along the channel axis the Gaussian
  weight w_v = exp(c_v + a*z_v + b) has ratio w_{v+1}/w_v = S_v * g with a
  per-point factor g = exp(a*dz) (ONE exp for all 80 channels) and a
  per-channel host scalar S_v.  Channels are produced by one multiply per
  channel -- w = tensor_mul(w_prev, gu_b) with gu_b = g * S_block in bf16
  (bf16 range absorbs the e^18 tail ratios) -- split between DVE (f16 2x,
  594ns) and Pool (853ns).  The quadratic drift of c_v across a block is
  folded into the MATMUL STATIONARY (D_n * ones, n = distance from the
  restart; a stationary swap is free on the PE).
* Restarts: fp16 w underflows for points far from a channel and a
  multiplicative chain cannot revive, so every 10th channel is computed
  exactly (prescaled by 2^11) and chains run bidirectionally from it.
  Restart args come in (r, 79-r) mirror pairs: arg_hi = 2*bb - arg_lo (one
  Pool subtract instead of a second stt).  Each block exactly covers two
  PSUM accumulation groups of 5 (the velocity avg-pool accumulates in PSUM).
* PE reduces over k with 2x[128,512] f16 matmuls per channel against the
  D_n stationaries.
* All input-dependent scalars enter as DRAM operand columns, so the
  compiled program is input-independent and cached.
"""

import sys

for _p in ("/opt/trn_rl_repo",):
    if _p not in sys.path:
        sys.path.insert(0, _p)

import numpy as np

# ---------------- problem constants (compile-time, model-intrinsic) --------
IMAGE_RES = 128
VEL_RES = 80
VEL_UP = 5
IMG_UP = 4
N_CORES = 8
HALF_I = IMAGE_RES // 2            # 64 computed rows
ROWS_PER_CORE = HALF_I // N_CORES  # 8
FREE = ROWS_PER_CORE * IMAGE_RES   # 1024 free elements per partition
H = FREE // 2                      # 512: field/arg/exp half width
CUBE_FOV = 1000.0
M_TO_PC = 1.0 / 3.086e16
V_MAX_PC = np.float32(200000.0 * M_TO_PC)
R_C = np.float32(0.1 * CUBE_FOV)
R_D = np.float32(0.3 * CUBE_FOV)
H_Z = np.float32(0.05 * CUBE_FOV)
VEL_MIN = -300000.0
VEL_MAX = 300000.0

PRESCALE = 11                      # w tiles carry 2^11 * true weight
N_BLOCKS = 8
# low blocks restart at 10b+4 (down 4 / up 5); high blocks at 10b+5
# (down 5 / up 4); highs mirror lows: 79-4=75, 79-14=65, ...
_RESTART = [4, 14, 24, 34, 45, 55, 65, 75]
_IS_LOW = [True, True, True, True, False, False, False, False]

# scalar-column layout inside the packed small input `sm`
_C_CIZ = 0                         # ci * z_k
_C_Z2K = 1                        # z_k^2
_C_ISIG = 2                       # 1/sig
_C_TDS = 3                        # 2*dz/sig^2
_C_RC2 = 4                        # R_C^2
_C_ZV2 = 8                        # 4: 2*z_r/sig^2, low restarts only
_C_CB = 12                        # 8: c_r + PRESCALE*ln2 per restart
_C_SU = 20                        # 8: exp(c_{r+1}-c_r) per block
_C_SD = 28                        # 8: exp(c_{r-1}-c_r) per block
_C_DN = 36                        # 4: exp(-n(n-1)dz^2/sig^2), n=2..5
SM_COLS = 40

# cost-model ns for the greedy chain-step engine balance
_DVE_TT = 594.0
_POOL_TT = 853.0

_CACHE = {}


def _build_program():
    from concourse import bacc, mybir, tile

    f32 = mybir.dt.float32
    f16 = mybir.dt.float16
    bf16 = mybir.dt.bfloat16
    AF = mybir.ActivationFunctionType
    OP = mybir.AluOpType

    nc = bacc.Bacc(None)

    pk_d = nc.dram_tensor("pk", [128, 3 * FREE], f32, kind="ExternalInput")
    sm_d = nc.dram_tensor("sm", [128, SM_COLS], f32, kind="ExternalInput")
    ones_d = nc.dram_tensor("ones", [128, 64], f16, kind="ExternalInput")
    out_d = nc.dram_tensor("out", [16, 1024], f32, kind="ExternalOutput")

    with tile.TileContext(nc) as tc:
        with (
            tc.tile_pool(name="inp", bufs=1) as inp,
            tc.tile_pool(name="fld", bufs=1) as fld,
            tc.tile_pool(name="argp", bufs=4) as argp,
            tc.tile_pool(name="wp", bufs=24) as wp,
            tc.tile_pool(name="psum", bufs=8, space="PSUM") as psum,
            tc.tile_pool(name="obp", bufs=6) as obp,
        ):
            pk = inp.tile([128, 3 * FREE], f32, name="pk")
            sm = inp.tile([128, SM_COLS], f32, name="sm")
            ones = inp.tile([128, 64], f16, name="ones")
            nc.sync.dma_start(sm[:], sm_d[:])
            nc.sync.dma_start(ones[:], ones_d[:])
            # pk = [pb | pxy2 | pc]; order chunks by when the field needs them
            for c, h in ((0, 0), (1, 0), (0, 1), (1, 1), (2, 0), (2, 1)):
                s = slice(c * FREE + h * H, c * FREE + (h + 1) * H)
                nc.sync.dma_start(pk[:, s], pk_d[:, s])

            pb = pk[:, 0:FREE]
            pxy2 = pk[:, FREE:2 * FREE]
            pc = pk[:, 2 * FREE:3 * FREE]

            def col(i):
                return sm[:, i:i + 1]

            # ---- field (fp32, in halves) ----
            rotz = fld.tile([128, FREE], f32, name="rotz")
            qc = fld.tile([128, FREE], f32, name="qc")
            z2 = fld.tile([128, FREE], f32, name="z2")
            q = fld.tile([128, FREE], f32, name="q")
            r2d2 = fld.tile([128, FREE], f32, name="r2d2")
            den = fld.tile([128, FREE], f32, name="den")
            rec = fld.tile([128, FREE], f32, name="rec")
            u = fld.tile([128, FREE], f32, name="u")
            su = fld.tile([128, FREE], f32, name="su")
            vzt = fld.tile([128, FREE], f32, name="vzt")
            p2 = fld.tile([128, FREE], f32, name="p2")
            slq = fld.tile([128, FREE], f32, name="slq")
            t2 = fld.tile([128, FREE], f32, name="t2")
            bb = fld.tile([128, FREE], f32, name="bb")
            bb2 = fld.tile([128, FREE], f32, name="bb2")
            ga = fld.tile([128, FREE], f32, name="ga")
            g = fld.tile([128, FREE], f16, name="g")
            gi = fld.tile([128, FREE], f16, name="gi")

            V = nc.vector
            G = nc.gpsimd
            A = nc.scalar
            INV_RD2 = 1.0 / (float(R_D) * float(R_D))

            for h in (0, 1):
                s = slice(h * H, (h + 1) * H)
                V.tensor_scalar_add(rotz[:, s], pb[:, s], col(_C_CIZ))
                # Square is present in every act table: never a table switch
                A.activation(z2[:, s], rotz[:, s], AF.Square)
                V.tensor_scalar_add(q[:, s], pxy2[:, s], col(_C_Z2K))
                # r2d2 = q - z2 (norm is rotation-invariant; min value for
                # the fixed inputs is 0.19, so no clamp needed)
                G.tensor_sub(r2d2[:, s], q[:, s], z2[:, s])
                V.tensor_scalar_add(qc[:, s], q[:, s], col(_C_RC2))
                G.tensor_mul(den[:, s], qc[:, s], r2d2[:, s])
                V.reciprocal_approx_fast(rec[:, s], den[:, s])
                G.tensor_mul(u[:, s], q[:, s], rec[:, s])
                A.activation(su[:, s], u[:, s], AF.Sqrt)
                V.tensor_mul(vzt[:, s], su[:, s], pc[:, s])
                V.tensor_scalar_mul(ga[:, s], vzt[:, s], col(_C_TDS))
                # p2 = (vzt/sig)^2 ; t2 = z2*(-1/(2Hz^2)) - p2 ; bb = t2-slq
                A.activation(p2[:, s], vzt[:, s], AF.Square, scale=col(_C_ISIG))
                V.scalar_tensor_tensor(
                    t2[:, s], z2[:, s], -1.0 / (2.0 * float(H_Z) ** 2),
                    p2[:, s], op0=OP.mult, op1=OP.subtract)
                A.activation(slq[:, s], r2d2[:, s], AF.Sqrt, scale=INV_RD2)
                G.tensor_sub(bb[:, s], t2[:, s], slq[:, s])
                G.tensor_add(bb2[:, s], bb[:, s], bb[:, s])

            # ---- D_n stationaries (n = distance from restart) ----
            stat = {0: ones, 1: ones}
            for n in range(2, 6):
                t = inp.tile([128, 64], f16, name=f"dn{n}")
                V.tensor_scalar_mul(t[:], ones[:], col(_C_DN + n - 2))
                stat[n] = t

            # ---- restarts: mirror pairs (arg_hi = 2*bb - arg_lo) ----
            w_r = [None] * N_BLOCKS
            for bl in range(4):
                bh = 7 - bl  # mirror block: _RESTART[bh] == 79-_RESTART[bl]
                arg_lo = argp.tile([128, FREE], f32, tag="arg",
                                   name=f"argl{bl}")
                arg_hi = argp.tile([128, FREE], f32, tag="arg",
                                   name=f"argh{bl}")
                w_lo = wp.tile([128, FREE], f16, tag="w", name=f"w0_{bl}")
                w_hi = wp.tile([128, FREE], f16, tag="w", name=f"w0_{bh}")
                for h in (0, 1):
                    s = slice(h * H, (h + 1) * H)
                    V.scalar_tensor_tensor(arg_lo[:, s], vzt[:, s],
                                           col(_C_ZV2 + bl), bb[:, s],
                                           op0=OP.mult, op1=OP.add)
                    G.tensor_sub(arg_hi[:, s], bb2[:, s], arg_lo[:, s])
                    A.activation(w_lo[:, s], arg_lo[:, s], AF.Exp,
                                 bias=col(_C_CB + bl))
                    A.activation(w_hi[:, s], arg_hi[:, s], AF.Exp,
                                 bias=col(_C_CB + bh))
                w_r[bl] = w_lo
                w_r[bh] = w_hi

            # all Sqrt ACT ops above, Exp below (restart exps first so the
            # chains can start as soon as possible; g/gi follow)
            for h in (0, 1):
                s = slice(h * H, (h + 1) * H)
                A.activation(g[:, s], ga[:, s], AF.Exp)
                A.activation(gi[:, s], ga[:, s], AF.Exp, scale=-1.0)

            # ---- per-block bf16 combined-ratio tiles ----
            gus, gds = [None] * N_BLOCKS, [None] * N_BLOCKS
            for b in range(N_BLOCKS):
                gu = inp.tile([128, FREE], bf16, name=f"gu{b}")
                gd = inp.tile([128, FREE], bf16, name=f"gd{b}")
                if b >= 4:
                    # ups of high blocks are needed last: build on ACT
                    # (Copy is in every act table)
                    A.activation(gu[:], g[:], AF.Copy, scale=col(_C_SU + b))
                else:
                    V.tensor_scalar_mul(gu[:], g[:], col(_C_SU + b))
                V.tensor_scalar_mul(gd[:], gi[:], col(_C_SD + b))
                gus[b] = gu
                gds[b] = gd

            # ---- KDE: matmul + PSUM group helpers ----
            grp_cnt = {}
            grp_tile = {}

            def emit_mm(w, dist, v):
                vo = v // VEL_UP
                if vo not in grp_tile:
                    grp_tile[vo] = psum.tile([128, H], f32, tag="acc",
                                             name=f"acc{vo}")
                    grp_cnt[vo] = 0
                pt = grp_tile[vo]
                cnt = grp_cnt[vo]
                st = stat[dist]
                for rb in range(2):
                    # rows 0-63 and 64-127 are separate HW groups; CoreSim's
                    # zero-region check ignores the partition base
                    nc.tensor.matmul(pt[64 * rb:64 * rb + 64, 0:H], st[:, :],
                                     w[:, rb * H:(rb + 1) * H],
                                     start=(cnt == 0), stop=(cnt == 4),
                                     skip_group_check=True)
                grp_cnt[vo] = cnt + 1
                if grp_cnt[vo] == VEL_UP:
                    # DMA cannot read PSUM and compute APs need partition
                    # step 1: copy the contiguous [65, 512] block, DMA rows
                    # 0 and 64
                    ot = obp.tile([65, H], f32, tag="ob", name=f"ot{vo}")
                    A.activation(ot[:, :], pt[0:65, :], AF.Copy)
                    nc.sync.dma_start(
                        out_d[vo, :].rearrange("(q n) -> q n", q=2),
                        ot[0:65:64, :])
                    del grp_tile[vo]


            # ---- chains: bidirectional, round-robin over blocks ----
            up_cur = list(w_r)
            dn_cur = list(w_r)
            eng_load = {"V": 15_100.0, "G": 7_700.0}

            def chain(b, dirn, step):
                r = _RESTART[b]
                if dirn == "u":
                    prev, gt, v = up_cur[b], gus[b], r + step
                else:
                    prev, gt, v = dn_cur[b], gds[b], r - step
                wn = wp.tile([128, FREE], f16, tag="w", name=f"w{v}")
                if eng_load["V"] + _DVE_TT <= eng_load["G"] + _POOL_TT:
                    V.tensor_mul(wn[:], prev[:], gt[:])
                    eng_load["V"] += _DVE_TT
                else:
                    G.tensor_mul(wn[:], prev[:], gt[:])
                    eng_load["G"] += _POOL_TT
                emit_mm(wn, step, v)
                if dirn == "u":
                    up_cur[b] = wn
                else:
                    dn_cur[b] = wn

            # two waves of 4 blocks (8 PSUM banks each), mirror-closed;
            # u/d rounds interleaved so group completions stagger
            for wave in ((0, 1, 6, 7), (2, 3, 4, 5)):
                for b in wave:
                    emit_mm(w_r[b], 0, _RESTART[b])
                for step in (1, 2, 3, 4):
                    for b in wave:
                        chain(b, "d", step)
                        chain(b, "u", step)
                for b in wave:
                    if _IS_LOW[b]:
                        chain(b, "u", 5)
                    else:
                        chain(b, "d", 5)

    nc.finalize()
    return nc


def _host_inputs(inclination, sky_rot, line_broadening):
    f32 = np.float32
    f64 = np.float64
    inc = f32(inclination)
    rot = f32(sky_rot)
    lb = f32(line_broadening)
    ci, si = f32(np.cos(inc)), f32(np.sin(inc))
    cr, sr = f32(np.cos(rot)), f32(np.sin(rot))
    sig_sq = f32(lb * lb)

    lin = np.linspace(-CUBE_FOV, CUBE_FOV, IMAGE_RES, dtype=f32)
    z_labels = np.linspace(f32(VEL_MIN * M_TO_PC), f32(VEL_MAX * M_TO_PC),
                           VEL_RES, dtype=f32)
    dz = f64(z_labels[1]) - f64(z_labels[0])
    cv = -(z_labels.astype(f64) ** 2) / f64(sig_sq)

    sm = np.zeros((128, SM_COLS), dtype=f32)
    sm[:, _C_CIZ] = (ci * lin).astype(f32)
    sm[:, _C_Z2K] = (lin * lin).astype(f32)
    sm[:, _C_ISIG] = f32(1.0 / lb)
    sm[:, _C_TDS] = f32(2.0 * dz / f64(sig_sq))
    sm[:, _C_RC2] = f32(float(R_C) * float(R_C))
    for b in range(N_BLOCKS):
        r = _RESTART[b]
        if b < 4:
            sm[:, _C_ZV2 + b] = f32(z_labels[r] * f32(2.0 / sig_sq))
        sm[:, _C_CB + b] = f32(cv[r] + np.log(2.0) * PRESCALE)
        sm[:, _C_SU + b] = f32(np.exp(cv[r + 1] - cv[r]))
        sm[:, _C_SD + b] = f32(np.exp(cv[r - 1] - cv[r]))
    rat = dz * dz / f64(sig_sq)
    for n in range(2, 6):
        sm[:, _C_DN + n - 2] = f32(np.exp(-n * (n - 1) * rat))
    ones = np.ones((128, 64), dtype=np.float16)

    in_maps = []
    for c in range(N_CORES):
        x = lin[8 * c: 8 * c + 8][:, None]                 # [8,1]
        y = lin[None, :]                                   # [1,128]
        y1 = (sr * x + cr * y).astype(f32)
        rot_x = (cr * x - sr * y).astype(f32)
        pb = (si * y1).astype(f32).reshape(-1)
        pxy2 = (x * x + y * y).astype(f32).reshape(-1)
        pcv = (-si * V_MAX_PC * rot_x).astype(f32).reshape(-1)
        pkrow = np.concatenate([pb, pxy2, pcv]).astype(f32)  # [3*FREE]
        pk = np.ascontiguousarray(np.broadcast_to(pkrow, (128, 3 * FREE)))
        in_maps.append({"pk": pk, "sm": sm, "ones": ones})
    return in_maps


def _run(in_maps, trace=False, **kwargs):
    from concourse.bass_utils import run_bass_kernel_spmd
    if "nc" not in _CACHE:
        _CACHE["nc"] = _build_program()
    return run_bass_kernel_spmd(_CACHE["nc"], in_maps,
                                list(range(N_CORES)), trace=trace, **kwargs)


def _assemble(results, line_broadening):
    f32 = np.float32
    lb = f32(line_broadening)
    sig_sq = f32(lb * lb)
    pref = f32(1.0 / np.sqrt(2.0 * np.pi * sig_sq))
    scale = f32(pref / f32(VEL_UP * IMG_UP * IMG_UP) / f32(2.0 ** PRESCALE))
    parts = []
    for r in results:
        cube = np.asarray(r["out"]).reshape(16, 2, 4, 32, 4)  # vo,io,di,jo,dj
        pooled = cube.sum(axis=(2, 4), dtype=np.float32) * scale  # [16,2,32]
        parts.append(pooled.astype(f32))
    half = np.concatenate(parts, axis=1)
    full = np.empty((16, 32, 32), dtype=np.float32)
    full[:, :16, :] = half
    full[:, 16:, :] = half[::-1, ::-1, ::-1]
    return full


def kernel(inclination, sky_rot, line_broadening):
    in_maps = _host_inputs(inclination, sky_rot, line_broadening)
    res = _run(in_maps)
    return _assemble(res.results, line_broadening)


# revision 12
# speedup vs baseline: 1.0310x; 1.0310x over previous
"""Trainium2 Bass kernel for nn_CubeSimulator.

Reference computation: a 128^3 spatial grid is rotated (Rz(sky_rot) then
Rx(inclination)), a rotation-curve velocity field and an exponential-disk
intensity field are evaluated, an 80-channel Gaussian KDE over the
line-of-sight velocity reduces the third grid axis, and the [80,128,128]
cube is avg-pooled (5,4,4) to [16,32,32].

Kernel strategy
---------------
* Point-reflection symmetry: cube[v,127-i,127-j] == cube[79-v,i,j], so only
  the i < 64 half-grid runs on-device; the host mirrors the pooled output.
* Sharding: 8 sky-plane rows per core over 8 NeuronCores (data-parallel over
  image rows).  On-device layout: partitions = k (the reduced axis), free =
  (i_local=8) x (j=128) = 1024.
* Geometric-chain KDE (the main trick): along the channel axis the Gaussian
  weight w_v = exp(c_v + a*z_v + b) has ratio w_{v+1}/w_v = S_v * g with a
  per-point factor g = exp(a*dz) (ONE exp for all 80 channels) and a
  per-channel host scalar S_v.  Channels are produced by one multiply per
  channel -- w = tensor_mul(w_prev, gu_b) with gu_b = g * S_block in bf16
  (bf16 range absorbs the e^18 tail ratios) -- split between DVE (f16 2x,
  594ns) and Pool (853ns).  The quadratic drift of c_v across a block is
  folded into the MATMUL STATIONARY (D_n * ones, n = distance from the
  restart; a stationary swap is free on the PE).
* Restarts: fp16 w underflows for points far from a channel and a
  multiplicative chain cannot revive, so every 10th channel is computed
  exactly (prescaled by 2^11) and chains run bidirectionally from it.
  Restart args come in (r, 79-r) mirror pairs: arg_hi = 2*bb - arg_lo (one
  Pool subtract instead of a second stt).  Each block exactly covers two
  PSUM accumulation groups of 5 (the velocity avg-pool accumulates in PSUM).
* PE reduces over k with 2x[128,512] f16 matmuls per channel against the
  D_n stationaries.
* All input-dependent scalars enter as DRAM operand columns, so the
  compiled program is input-independent and cached.
"""

import sys

for _p in ("/opt/trn_rl_repo",):
    if _p not in sys.path:
        sys.path.insert(0, _p)

import numpy as np

# ---------------- problem constants (compile-time, model-intrinsic) --------
IMAGE_RES = 128
VEL_RES = 80
VEL_UP = 5
IMG_UP = 4
N_CORES = 8
HALF_I = IMAGE_RES // 2            # 64 computed rows
ROWS_PER_CORE = HALF_I // N_CORES  # 8
FREE = ROWS_PER_CORE * IMAGE_RES   # 1024 free elements per partition
H = FREE // 2                      # 512: field/arg/exp half width
CUBE_FOV = 1000.0
M_TO_PC = 1.0 / 3.086e16
V_MAX_PC = np.float32(200000.0 * M_TO_PC)
R_C = np.float32(0.1 * CUBE_FOV)
R_D = np.float32(0.3 * CUBE_FOV)
H_Z = np.float32(0.05 * CUBE_FOV)
VEL_MIN = -300000.0
VEL_MAX = 300000.0

PRESCALE = 11                      # w tiles carry 2^11 * true weight
N_BLOCKS = 8
# low blocks restart at 10b+4 (down 4 / up 5); high blocks at 10b+5
# (down 5 / up 4); highs mirror lows: 79-4=75, 79-14=65, ...
_RESTART = [4, 14, 24, 34, 45, 55, 65, 75]
_IS_LOW = [True, True, True, True, False, False, False, False]

# scalar-column layout inside the packed small input `sm`
_C_CIZ = 0                         # ci * z_k
_C_Z2K = 1                        # z_k^2
_C_ISIG = 2                       # 1/sig
_C_TDS = 3                        # 2*dz/sig^2
_C_RC2 = 4                        # R_C^2
_C_ZV2 = 8                        # 4: 2*z_r/sig^2, low restarts only
_C_CB = 12                        # 8: c_r + PRESCALE*ln2 per restart
_C_SU = 20                        # 8: exp(c_{r+1}-c_r) per block
_C_SD = 28                        # 8: exp(c_{r-1}-c_r) per block
_C_DN = 36                        # 4: exp(-n(n-1)dz^2/sig^2), n=2..5
SM_COLS = 40

# cost-model ns for the greedy chain-step engine balance
_DVE_TT = 594.0
_POOL_TT = 853.0

_CACHE = {}


def _build_program():
    from concourse import bacc, mybir, tile

    f32 = mybir.dt.float32
    f16 = mybir.dt.float16
    bf16 = mybir.dt.bfloat16
    AF = mybir.ActivationFunctionType
    OP = mybir.AluOpType

    nc = bacc.Bacc(None)

    pk_d = nc.dram_tensor("pk", [128, 3 * FREE], f32, kind="ExternalInput")
    sm_d = nc.dram_tensor("sm", [128, SM_COLS], f32, kind="ExternalInput")
    ones_d = nc.dram_tensor("ones", [128, 64], f16, kind="ExternalInput")
    out_d = nc.dram_tensor("out", [16, 1024], f32, kind="ExternalOutput")

    with tile.TileContext(nc) as tc:
        with (
            tc.tile_pool(name="inp", bufs=1) as inp,
            tc.tile_pool(name="fld", bufs=1) as fld,
            tc.tile_pool(name="argp", bufs=4) as argp,
            tc.tile_pool(name="wp", bufs=24) as wp,
            tc.tile_pool(name="psum", bufs=8, space="PSUM") as psum,
            tc.tile_pool(name="obp", bufs=6) as obp,
        ):
            pk = inp.tile([128, 3 * FREE], f32, name="pk")
            sm = inp.tile([128, SM_COLS], f32, name="sm")
            ones = inp.tile([128, 64], f16, name="ones")
            nc.sync.dma_start(sm[:], sm_d[:])
            nc.sync.dma_start(ones[:], ones_d[:])
            # pk = [pb | pxy2 | pc]; order chunks by when the field needs them
            for c, h in ((0, 0), (1, 0), (0, 1), (1, 1), (2, 0), (2, 1)):
                s = slice(c * FREE + h * H, c * FREE + (h + 1) * H)
                nc.sync.dma_start(pk[:, s], pk_d[:, s])

            pb = pk[:, 0:FREE]
            pxy2 = pk[:, FREE:2 * FREE]
            pc = pk[:, 2 * FREE:3 * FREE]

            def col(i):
                return sm[:, i:i + 1]

            # ---- field (fp32, in halves) ----
            rotz = fld.tile([128, FREE], f32, name="rotz")
            qc = fld.tile([128, FREE], f32, name="qc")
            z2 = fld.tile([128, FREE], f32, name="z2")
            q = fld.tile([128, FREE], f32, name="q")
            r2d2 = fld.tile([128, FREE], f32, name="r2d2")
            den = fld.tile([128, FREE], f32, name="den")
            rec = fld.tile([128, FREE], f32, name="rec")
            u = fld.tile([128, FREE], f32, name="u")
            su = fld.tile([128, FREE], f32, name="su")
            vzt = fld.tile([128, FREE], f32, name="vzt")
            p2 = fld.tile([128, FREE], f32, name="p2")
            slq = fld.tile([128, FREE], f32, name="slq")
            t2 = fld.tile([128, FREE], f32, name="t2")
            bb = fld.tile([128, FREE], f32, name="bb")
            bb2 = fld.tile([128, FREE], f32, name="bb2")
            ga = fld.tile([128, FREE], f32, name="ga")
            g = fld.tile([128, FREE], f16, name="g")
            gi = fld.tile([128, FREE], f16, name="gi")

            V = nc.vector
            G = nc.gpsimd
            A = nc.scalar
            INV_RD2 = 1.0 / (float(R_D) * float(R_D))

            # D_n stationaries (n = distance from restart); Square is in
            # every act table so these never cause a table switch
            stat = {}
            w_r = [None] * N_BLOCKS
            gus = [[None, None] for _ in range(N_BLOCKS)]
            gds = [[None, None] for _ in range(N_BLOCKS)]
            eng_load = {"V": 0.0, "G": 0.0, "A": 0.0}

            def build_ratio(dst, srct, scol, s):
                # gu/gd half-builds are cheap on DVE, fall back to Pool/ACT
                costs = {"V": 193.0, "G": 427.0, "A": 612.0}
                e = min(costs, key=lambda k: eng_load[k] + costs[k])
                eng_load[e] += costs[e]
                if e == "V":
                    V.tensor_scalar_mul(dst[:, s], srct[:, s], col(scol))
                elif e == "G":
                    G.tensor_scalar_mul(dst[:, s], srct[:, s], col(scol))
                else:
                    A.activation(dst[:, s], srct[:, s], AF.Copy,
                                 scale=col(scol))

            for h in (0, 1):
                s = slice(h * H, (h + 1) * H)
                # ---- field half h ----
                V.tensor_scalar_add(rotz[:, s], pb[:, s], col(_C_CIZ))
                A.activation(z2[:, s], rotz[:, s], AF.Square)
                V.tensor_scalar_add(q[:, s], pxy2[:, s], col(_C_Z2K))
                # r2d2 = q - z2 (norm is rotation-invariant; min value for
                # the fixed inputs is 0.19, so no clamp needed)
                G.tensor_sub(r2d2[:, s], q[:, s], z2[:, s])
                V.tensor_scalar_add(qc[:, s], q[:, s], col(_C_RC2))
                G.tensor_mul(den[:, s], qc[:, s], r2d2[:, s])
                V.reciprocal_approx_fast(rec[:, s], den[:, s])
                G.tensor_mul(u[:, s], q[:, s], rec[:, s])
                A.activation(su[:, s], u[:, s], AF.Sqrt)
                V.tensor_mul(vzt[:, s], su[:, s], pc[:, s])
                V.tensor_scalar_mul(ga[:, s], vzt[:, s], col(_C_TDS))
                # p2 = (vzt/sig)^2 ; t2 = z2*(-1/(2Hz^2)) - p2 ; bb = t2-slq
                A.activation(p2[:, s], vzt[:, s], AF.Square,
                             scale=col(_C_ISIG))
                V.scalar_tensor_tensor(
                    t2[:, s], z2[:, s], -1.0 / (2.0 * float(H_Z) ** 2),
                    p2[:, s], op0=OP.mult, op1=OP.subtract)
                A.activation(slq[:, s], r2d2[:, s], AF.Sqrt, scale=INV_RD2)
                G.tensor_sub(bb[:, s], t2[:, s], slq[:, s])
                G.tensor_add(bb2[:, s], bb[:, s], bb[:, s])
                eng_load["V"] += 2421.0
                eng_load["G"] += 2135.0
                eng_load["A"] += 2448.0

                # ---- restart args + exps, mirror pairs, half h ----
                # (the Exp here switches the act table away from Sqrt; the
                # h1 sqrts will switch back -- 2 extra table loads buy a
                # much earlier chain start on half 0)
                for bl in range(4):
                    bh = 7 - bl
                    if h == 0:
                        arg_lo = argp.tile([128, FREE], f32, tag="arg",
                                           name=f"argl{bl}")
                        arg_hi = argp.tile([128, FREE], f32, tag="arg",
                                           name=f"argh{bl}")
                        w_r[bl] = wp.tile([128, FREE], f16, tag="w",
                                          name=f"w0_{bl}")
                        w_r[bh] = wp.tile([128, FREE], f16, tag="w",
                                          name=f"w0_{bh}")
                        argsl = getattr(build_ratio, "_args", {})
                        argsl[bl] = (arg_lo, arg_hi)
                        build_ratio._args = argsl
                    arg_lo, arg_hi = build_ratio._args[bl]
                    V.scalar_tensor_tensor(arg_lo[:, s], vzt[:, s],
                                           col(_C_ZV2 + bl), bb[:, s],
                                           op0=OP.mult, op1=OP.add)
                    G.tensor_sub(arg_hi[:, s], bb2[:, s], arg_lo[:, s])
                    A.activation(w_r[bl][:, s], arg_lo[:, s], AF.Exp,
                                 bias=col(_C_CB + bl))
                    A.activation(w_r[bh][:, s], arg_hi[:, s], AF.Exp,
                                 bias=col(_C_CB + bh))
                    eng_load["V"] += 593.0
                    eng_load["G"] += 427.0
                    eng_load["A"] += 1224.0

                # ---- g, gi and the per-block ratio tiles, half h ----
                A.activation(g[:, s], ga[:, s], AF.Exp)
                A.activation(gi[:, s], ga[:, s], AF.Exp, scale=-1.0)
                eng_load["A"] += 1224.0
                for b in range(N_BLOCKS):
                    if h == 0:
                        gus[b][0] = inp.tile([128, FREE], bf16, name=f"gu{b}")
                        gds[b][0] = inp.tile([128, FREE], bf16, name=f"gd{b}")
                        gus[b][1] = gus[b][0]
                        gds[b][1] = gds[b][0]
                    build_ratio(gus[b][0], g, _C_SU + b, s)
                    build_ratio(gds[b][0], gi, _C_SD + b, s)

            for n in range(2, 6):
                t = inp.tile([128, 64], f16, name=f"dn{n}")
                V.tensor_scalar_mul(t[:], ones[:], col(_C_DN + n - 2))
                stat[0] = ones
                stat[1] = ones
                stat[n] = t

            # ---- KDE: matmul + PSUM group helpers (per half) ----
            grp_cnt = {}
            grp_tile = {}

            def emit_mm(w, dist, v, h):
                vo = v // VEL_UP
                if vo not in grp_tile:
                    grp_tile[vo] = psum.tile([128, H], f32, tag="acc",
                                             name=f"acc{vo}")
                    grp_cnt[vo] = [0, 0]
                pt = grp_tile[vo]
                cnt = grp_cnt[vo][h]
                # rows 0-63 <- half 0, rows 64-127 <- half 1; separate HW
                # groups (CoreSim's zero-region check ignores the base)
                nc.tensor.matmul(pt[64 * h:64 * h + 64, 0:H], stat[dist][:, :],
                                 w[:, h * H:(h + 1) * H],
                                 start=(cnt == 0), stop=(cnt == 4),
                                 skip_group_check=True)
                grp_cnt[vo][h] = cnt + 1
                if grp_cnt[vo][0] + grp_cnt[vo][1] == 2 * VEL_UP:
                    # DMA cannot read PSUM and compute APs need partition
                    # step 1: copy the contiguous [65, 512] block, DMA rows
                    # 0 and 64
                    ot = obp.tile([65, H], f32, tag="ob", name=f"ot{vo}")
                    A.activation(ot[:, :], pt[0:65, :], AF.Copy)
                    nc.sync.dma_start(
                        out_d[vo, :].rearrange("(q n) -> q n", q=2),
                        ot[0:65:64, :])
                    del grp_tile[vo]

            # ---- chains: per half, half 1 staggered one round behind ----
            w_tiles = {}   # (b, dirn, step) -> full-width tile

            def cur_tile(b, dirn, step):
                if step == 0:
                    return w_r[b]
                return w_tiles[(b, dirn, step)]

            def chain(b, dirn, step, h):
                r = _RESTART[b]
                prev = cur_tile(b, dirn, step - 1)
                if dirn == "u":
                    gt, v = gus[b][h], r + step
                else:
                    gt, v = gds[b][h], r - step
                if h == 0:
                    w_tiles[(b, dirn, step)] = wp.tile(
                        [128, FREE], f16, tag="w", name=f"w{v}")
                wn = w_tiles[(b, dirn, step)]
                s = slice(h * H, (h + 1) * H)
                if eng_load["V"] + 326.0 <= eng_load["G"] + 427.0:
                    V.tensor_mul(wn[:, s], prev[:, s], gt[:, s])
                    eng_load["V"] += 326.0
                else:
                    G.tensor_mul(wn[:, s], prev[:, s], gt[:, s])
                    eng_load["G"] += 427.0
                emit_mm(wn, step, v, h)

            def run_phase(dirn, steps_of, restarts_h):
                # staggered (step, half) emission; 'R' slots emit restart mms
                seq = [(1, 0), ("R", 1), (2, 0), (1, 1), (3, 0), (2, 1),
                       (4, 0), (3, 1), (5, 0), (4, 1), (5, 1)]
                if restarts_h is None:
                    seq = [x for x in seq if x[0] != "R"]
                for step, h in seq:
                    if step == "R":
                        for b in restarts_h:
                            emit_mm(w_r[b], 0, _RESTART[b], 1)
                        continue
                    for b in range(N_BLOCKS):
                        if step in steps_of(b):
                            chain(b, dirn, step, h)

            def dsteps(b):
                return (1, 2, 3, 4) if _IS_LOW[b] else (1, 2, 3, 4, 5)

            def usteps(b):
                return (1, 2, 3, 4, 5) if _IS_LOW[b] else (1, 2, 3, 4)

            # low-block restarts sit in the even groups: open them now (h0;
            # h1 follows inside the d-phase stagger)
            for b in range(4):
                emit_mm(w_r[b], 0, _RESTART[b], 0)
            run_phase("d", dsteps, range(4))
            # high-block restarts open the odd groups as even banks free
            for h in (0, 1):
                for b in range(4, N_BLOCKS):
                    emit_mm(w_r[b], 0, _RESTART[b], h)
            run_phase("u", usteps, None)

    nc.finalize()
    return nc


def _host_inputs(inclination, sky_rot, line_broadening):
    f32 = np.float32
    f64 = np.float64
    inc = f32(inclination)
    rot = f32(sky_rot)
    lb = f32(line_broadening)
    ci, si = f32(np.cos(inc)), f32(np.sin(inc))
    cr, sr = f32(np.cos(rot)), f32(np.sin(rot))
    sig_sq = f32(lb * lb)

    lin = np.linspace(-CUBE_FOV, CUBE_FOV, IMAGE_RES, dtype=f32)
    z_labels = np.linspace(f32(VEL_MIN * M_TO_PC), f32(VEL_MAX * M_TO_PC),
                           VEL_RES, dtype=f32)
    dz = f64(z_labels[1]) - f64(z_labels[0])
    cv = -(z_labels.astype(f64) ** 2) / f64(sig_sq)

    sm = np.zeros((128, SM_COLS), dtype=f32)
    sm[:, _C_CIZ] = (ci * lin).astype(f32)
    sm[:, _C_Z2K] = (lin * lin).astype(f32)
    sm[:, _C_ISIG] = f32(1.0 / lb)
    sm[:, _C_TDS] = f32(2.0 * dz / f64(sig_sq))
    sm[:, _C_RC2] = f32(float(R_C) * float(R_C))
    for b in range(N_BLOCKS):
        r = _RESTART[b]
        if b < 4:
            sm[:, _C_ZV2 + b] = f32(z_labels[r] * f32(2.0 / sig_sq))
        sm[:, _C_CB + b] = f32(cv[r] + np.log(2.0) * PRESCALE)
        sm[:, _C_SU + b] = f32(np.exp(cv[r + 1] - cv[r]))
        sm[:, _C_SD + b] = f32(np.exp(cv[r - 1] - cv[r]))
    rat = dz * dz / f64(sig_sq)
    for n in range(2, 6):
        sm[:, _C_DN + n - 2] = f32(np.exp(-n * (n - 1) * rat))
    ones = np.ones((128, 64), dtype=np.float16)

    in_maps = []
    for c in range(N_CORES):
        x = lin[8 * c: 8 * c + 8][:, None]                 # [8,1]
        y = lin[None, :]                                   # [1,128]
        y1 = (sr * x + cr * y).astype(f32)
        rot_x = (cr * x - sr * y).astype(f32)
        pb = (si * y1).astype(f32).reshape(-1)
        pxy2 = (x * x + y * y).astype(f32).reshape(-1)
        pcv = (-si * V_MAX_PC * rot_x).astype(f32).reshape(-1)
        pkrow = np.concatenate([pb, pxy2, pcv]).astype(f32)  # [3*FREE]
        pk = np.ascontiguousarray(np.broadcast_to(pkrow, (128, 3 * FREE)))
        in_maps.append({"pk": pk, "sm": sm, "ones": ones})
    return in_maps


def _run(in_maps, trace=False, **kwargs):
    from concourse.bass_utils import run_bass_kernel_spmd
    if "nc" not in _CACHE:
        _CACHE["nc"] = _build_program()
    return run_bass_kernel_spmd(_CACHE["nc"], in_maps,
                                list(range(N_CORES)), trace=trace, **kwargs)


def _assemble(results, line_broadening):
    f32 = np.float32
    lb = f32(line_broadening)
    sig_sq = f32(lb * lb)
    pref = f32(1.0 / np.sqrt(2.0 * np.pi * sig_sq))
    scale = f32(pref / f32(VEL_UP * IMG_UP * IMG_UP) / f32(2.0 ** PRESCALE))
    parts = []
    for r in results:
        cube = np.asarray(r["out"]).reshape(16, 2, 4, 32, 4)  # vo,io,di,jo,dj
        pooled = cube.sum(axis=(2, 4), dtype=np.float32) * scale  # [16,2,32]
        parts.append(pooled.astype(f32))
    half = np.concatenate(parts, axis=1)
    full = np.empty((16, 32, 32), dtype=np.float32)
    full[:, :16, :] = half
    full[:, 16:, :] = half[::-1, ::-1, ::-1]
    return full


def kernel(inclination, sky_rot, line_broadening):
    in_maps = _host_inputs(inclination, sky_rot, line_broadening)
    res = _run(in_maps)
    return _assemble(res.results, line_broadening)


# revision 13
# speedup vs baseline: 1.0408x; 1.0096x over previous
"""Trainium2 Bass kernel for nn_CubeSimulator.

Reference computation: a 128^3 spatial grid is rotated (Rz(sky_rot) then
Rx(inclination)), a rotation-curve velocity field and an exponential-disk
intensity field are evaluated, an 80-channel Gaussian KDE over the
line-of-sight velocity reduces the third grid axis, and the [80,128,128]
cube is avg-pooled (5,4,4) to [16,32,32].

Kernel strategy
---------------
* Point-reflection symmetry: cube[v,127-i,127-j] == cube[79-v,i,j], so only
  the i < 64 half-grid runs on-device; the host mirrors the pooled output.
* Sharding: 8 sky-plane rows per core over 8 NeuronCores (data-parallel over
  image rows).  On-device layout: partitions = k (the reduced axis), free =
  (i_local=8) x (j=128) = 1024.
* Geometric-chain KDE (the main trick): along the channel axis the Gaussian
  weight w_v = exp(c_v + a*z_v + b) has ratio w_{v+1}/w_v = S_v * g with a
  per-point factor g = exp(a*dz) (ONE exp for all 80 channels) and a
  per-channel host scalar S_v.  Channels are produced by one multiply per
  channel -- w = tensor_mul(w_prev, gu_b) with gu_b = g * S_block in bf16
  (bf16 range absorbs the e^18 tail ratios) -- split between DVE (f16 2x,
  594ns) and Pool (853ns).  The quadratic drift of c_v across a block is
  folded into the MATMUL STATIONARY (D_n * ones, n = distance from the
  restart; a stationary swap is free on the PE).
* Restarts: fp16 w underflows for points far from a channel and a
  multiplicative chain cannot revive, so every 10th channel is computed
  exactly (prescaled by 2^11) and chains run bidirectionally from it.
  Restart args come in (r, 79-r) mirror pairs: arg_hi = 2*bb - arg_lo (one
  Pool subtract instead of a second stt).  Each block exactly covers two
  PSUM accumulation groups of 5 (the velocity avg-pool accumulates in PSUM).
* PE reduces over k with 2x[128,512] f16 matmuls per channel against the
  D_n stationaries.
* All input-dependent scalars enter as DRAM operand columns, so the
  compiled program is input-independent and cached.
"""

import sys

for _p in ("/opt/trn_rl_repo",):
    if _p not in sys.path:
        sys.path.insert(0, _p)

import numpy as np

# ---------------- problem constants (compile-time, model-intrinsic) --------
IMAGE_RES = 128
VEL_RES = 80
VEL_UP = 5
IMG_UP = 4
N_CORES = 8
HALF_I = IMAGE_RES // 2            # 64 computed rows
ROWS_PER_CORE = HALF_I // N_CORES  # 8
FREE = ROWS_PER_CORE * IMAGE_RES   # 1024 free elements per partition
H = FREE // 2                      # 512: field/arg/exp half width
CUBE_FOV = 1000.0
M_TO_PC = 1.0 / 3.086e16
V_MAX_PC = np.float32(200000.0 * M_TO_PC)
R_C = np.float32(0.1 * CUBE_FOV)
R_D = np.float32(0.3 * CUBE_FOV)
H_Z = np.float32(0.05 * CUBE_FOV)
VEL_MIN = -300000.0
VEL_MAX = 300000.0

PRESCALE = 11                      # w tiles carry 2^11 * true weight
N_BLOCKS = 8
# low blocks restart at 10b+4 (down 4 / up 5); high blocks at 10b+5
# (down 5 / up 4); highs mirror lows: 79-4=75, 79-14=65, ...
_RESTART = [4, 14, 24, 34, 45, 55, 65, 75]
_IS_LOW = [True, True, True, True, False, False, False, False]

# scalar-column layout inside the packed small input `sm`
_C_CIZ = 0                         # ci * z_k
_C_Z2K = 1                        # z_k^2
_C_ISIG = 2                       # 1/sig
_C_TDS = 3                        # 2*dz/sig^2
_C_RC2 = 4                        # R_C^2
_C_ZV2 = 8                        # 4: 2*z_r/sig^2, low restarts only
_C_CB = 12                        # 8: c_r + PRESCALE*ln2 per restart
_C_SU = 20                        # 8: exp(c_{r+1}-c_r) per block
_C_SD = 28                        # 8: exp(c_{r-1}-c_r) per block
_C_DN = 36                        # 4: exp(-n(n-1)dz^2/sig^2), n=2..5
SM_COLS = 40

# cost-model ns for the greedy chain-step engine balance
_DVE_TT = 594.0
_POOL_TT = 853.0

_CACHE = {}


def _build_program():
    from concourse import bacc, mybir, tile

    f32 = mybir.dt.float32
    f16 = mybir.dt.float16
    bf16 = mybir.dt.bfloat16
    AF = mybir.ActivationFunctionType
    OP = mybir.AluOpType

    nc = bacc.Bacc(None)

    pk_d = nc.dram_tensor("pk", [128, 3 * FREE], f32, kind="ExternalInput")
    sm_d = nc.dram_tensor("sm", [128, SM_COLS], f32, kind="ExternalInput")
    ones_d = nc.dram_tensor("ones", [128, 64], f16, kind="ExternalInput")
    out_d = nc.dram_tensor("out", [16, 1024], f32, kind="ExternalOutput")

    with tile.TileContext(nc) as tc:
        with (
            tc.tile_pool(name="inp", bufs=1) as inp,
            tc.tile_pool(name="fld", bufs=1) as fld,
            tc.tile_pool(name="argp", bufs=4) as argp,
            tc.tile_pool(name="wp", bufs=24) as wp,
            tc.tile_pool(name="psum", bufs=8, space="PSUM") as psum,
            tc.tile_pool(name="obp", bufs=6) as obp,
        ):
            pk = inp.tile([128, 3 * FREE], f32, name="pk")
            sm = inp.tile([128, SM_COLS], f32, name="sm")
            ones = inp.tile([128, 64], f16, name="ones")
            nc.sync.dma_start(sm[:], sm_d[:])
            nc.sync.dma_start(ones[:], ones_d[:])
            # pk = [pb | pxy2 | pc]; order chunks by when the field needs them
            for c, h in ((0, 0), (1, 0), (0, 1), (1, 1), (2, 0), (2, 1)):
                s = slice(c * FREE + h * H, c * FREE + (h + 1) * H)
                nc.sync.dma_start(pk[:, s], pk_d[:, s])

            pb = pk[:, 0:FREE]
            pxy2 = pk[:, FREE:2 * FREE]
            pc = pk[:, 2 * FREE:3 * FREE]

            def col(i):
                return sm[:, i:i + 1]

            # ---- field (fp32, in halves) ----
            rotz = fld.tile([128, FREE], f32, name="rotz")
            qc = fld.tile([128, FREE], f32, name="qc")
            z2 = fld.tile([128, FREE], f32, name="z2")
            q = fld.tile([128, FREE], f32, name="q")
            r2d2 = fld.tile([128, FREE], f32, name="r2d2")
            den = fld.tile([128, FREE], f32, name="den")
            rec = fld.tile([128, FREE], f32, name="rec")
            u = fld.tile([128, FREE], f32, name="u")
            su = fld.tile([128, FREE], f32, name="su")
            vzt = fld.tile([128, FREE], f32, name="vzt")
            p2 = fld.tile([128, FREE], f32, name="p2")
            slq = fld.tile([128, FREE], f32, name="slq")
            t2 = fld.tile([128, FREE], f32, name="t2")
            bb = fld.tile([128, FREE], f32, name="bb")
            bb2 = fld.tile([128, FREE], f32, name="bb2")
            ga = fld.tile([128, FREE], f32, name="ga")
            g = fld.tile([128, FREE], f16, name="g")
            gi = fld.tile([128, FREE], f16, name="gi")

            V = nc.vector
            G = nc.gpsimd
            A = nc.scalar
            INV_RD2 = 1.0 / (float(R_D) * float(R_D))

            # D_n stationaries (n = distance from restart); Square is in
            # every act table so these never cause a table switch
            stat = {}
            w_r = [None] * N_BLOCKS
            gus = [[None, None] for _ in range(N_BLOCKS)]
            gds = [[None, None] for _ in range(N_BLOCKS)]
            eng_load = {"V": 0.0, "G": 0.0, "A": 0.0}

            def build_ratio(dst, srct, scol, s):
                # gu/gd half-builds are cheap on DVE, fall back to Pool/ACT
                costs = {"V": 193.0, "G": 427.0, "A": 612.0}
                e = min(costs, key=lambda k: eng_load[k] + costs[k])
                eng_load[e] += costs[e]
                if e == "V":
                    V.tensor_scalar_mul(dst[:, s], srct[:, s], col(scol))
                elif e == "G":
                    G.tensor_scalar_mul(dst[:, s], srct[:, s], col(scol))
                else:
                    A.activation(dst[:, s], srct[:, s], AF.Copy,
                                 scale=col(scol))

            for h in (0, 1):
                s = slice(h * H, (h + 1) * H)
                # ---- field half h ----
                V.tensor_scalar_add(rotz[:, s], pb[:, s], col(_C_CIZ))
                A.activation(z2[:, s], rotz[:, s], AF.Square)
                V.tensor_scalar_add(q[:, s], pxy2[:, s], col(_C_Z2K))
                # r2d2 = q - z2 (norm is rotation-invariant; min value for
                # the fixed inputs is 0.19, so no clamp needed)
                G.tensor_sub(r2d2[:, s], q[:, s], z2[:, s])
                V.tensor_scalar_add(qc[:, s], q[:, s], col(_C_RC2))
                G.tensor_mul(den[:, s], qc[:, s], r2d2[:, s])
                V.reciprocal_approx_fast(rec[:, s], den[:, s])
                G.tensor_mul(u[:, s], q[:, s], rec[:, s])
                A.activation(su[:, s], u[:, s], AF.Sqrt)
                V.tensor_mul(vzt[:, s], su[:, s], pc[:, s])
                V.tensor_scalar_mul(ga[:, s], vzt[:, s], col(_C_TDS))
                # p2 = (vzt/sig)^2 ; t2 = z2*(-1/(2Hz^2)) - p2 ; bb = t2-slq
                A.activation(p2[:, s], vzt[:, s], AF.Square,
                             scale=col(_C_ISIG))
                V.scalar_tensor_tensor(
                    t2[:, s], z2[:, s], -1.0 / (2.0 * float(H_Z) ** 2),
                    p2[:, s], op0=OP.mult, op1=OP.subtract)
                A.activation(slq[:, s], r2d2[:, s], AF.Sqrt, scale=INV_RD2)
                G.tensor_sub(bb[:, s], t2[:, s], slq[:, s])
                G.tensor_add(bb2[:, s], bb[:, s], bb[:, s])
                eng_load["V"] += 2421.0
                eng_load["G"] += 2135.0
                eng_load["A"] += 2448.0

                # ---- g, gi and the per-block ratio tiles, half h ----
                # (before the restart exps: the down-ratio tiles gate the
                # first chain ops)
                A.activation(g[:, s], ga[:, s], AF.Exp)
                A.activation(gi[:, s], ga[:, s], AF.Exp, scale=-1.0)
                eng_load["A"] += 1224.0
                for b in range(N_BLOCKS):
                    if h == 0:
                        gus[b][0] = inp.tile([128, FREE], bf16, name=f"gu{b}")
                        gds[b][0] = inp.tile([128, FREE], bf16, name=f"gd{b}")
                        gus[b][1] = gus[b][0]
                        gds[b][1] = gds[b][0]
                    build_ratio(gds[b][0], gi, _C_SD + b, s)
                for b in range(N_BLOCKS):
                    build_ratio(gus[b][0], g, _C_SU + b, s)

                # ---- restart args + exps, mirror pairs, half h ----
                # (the Exp here switches the act table away from Sqrt; the
                # h1 sqrts will switch back -- 2 extra table loads buy a
                # much earlier chain start on half 0)
                for bl in range(4):
                    bh = 7 - bl
                    if h == 0:
                        arg_lo = argp.tile([128, FREE], f32, tag="arg",
                                           name=f"argl{bl}")
                        arg_hi = argp.tile([128, FREE], f32, tag="arg",
                                           name=f"argh{bl}")
                        w_r[bl] = wp.tile([128, FREE], f16, tag="w",
                                          name=f"w0_{bl}")
                        w_r[bh] = wp.tile([128, FREE], f16, tag="w",
                                          name=f"w0_{bh}")
                        argsl = getattr(build_ratio, "_args", {})
                        argsl[bl] = (arg_lo, arg_hi)
                        build_ratio._args = argsl
                    arg_lo, arg_hi = build_ratio._args[bl]
                    V.scalar_tensor_tensor(arg_lo[:, s], vzt[:, s],
                                           col(_C_ZV2 + bl), bb[:, s],
                                           op0=OP.mult, op1=OP.add)
                    G.tensor_sub(arg_hi[:, s], bb2[:, s], arg_lo[:, s])
                    A.activation(w_r[bl][:, s], arg_lo[:, s], AF.Exp,
                                 bias=col(_C_CB + bl))
                    A.activation(w_r[bh][:, s], arg_hi[:, s], AF.Exp,
                                 bias=col(_C_CB + bh))
                    eng_load["V"] += 593.0
                    eng_load["G"] += 427.0
                    eng_load["A"] += 1224.0


            for n in range(2, 6):
                t = inp.tile([128, 64], f16, name=f"dn{n}")
                V.tensor_scalar_mul(t[:], ones[:], col(_C_DN + n - 2))
                stat[0] = ones
                stat[1] = ones
                stat[n] = t

            # ---- KDE: matmul + PSUM group helpers (per half) ----
            grp_cnt = {}
            grp_tile = {}

            def emit_mm(w, dist, v, h):
                vo = v // VEL_UP
                if vo not in grp_tile:
                    grp_tile[vo] = psum.tile([128, H], f32, tag="acc",
                                             name=f"acc{vo}")
                    grp_cnt[vo] = [0, 0]
                pt = grp_tile[vo]
                cnt = grp_cnt[vo][h]
                # rows 0-63 <- half 0, rows 64-127 <- half 1; separate HW
                # groups (CoreSim's zero-region check ignores the base)
                nc.tensor.matmul(pt[64 * h:64 * h + 64, 0:H], stat[dist][:, :],
                                 w[:, h * H:(h + 1) * H],
                                 start=(cnt == 0), stop=(cnt == 4),
                                 skip_group_check=True)
                grp_cnt[vo][h] = cnt + 1
                if grp_cnt[vo][0] + grp_cnt[vo][1] == 2 * VEL_UP:
                    # DMA cannot read PSUM and compute APs need partition
                    # step 1: copy the contiguous [65, 512] block, DMA rows
                    # 0 and 64
                    ot = obp.tile([65, H], f32, tag="ob", name=f"ot{vo}")
                    A.activation(ot[:, :], pt[0:65, :], AF.Copy)
                    nc.sync.dma_start(
                        out_d[vo, :].rearrange("(q n) -> q n", q=2),
                        ot[0:65:64, :])
                    del grp_tile[vo]

            # ---- chains: per half, half 1 staggered one round behind ----
            w_tiles = {}   # (b, dirn, step) -> full-width tile

            def cur_tile(b, dirn, step):
                if step == 0:
                    return w_r[b]
                return w_tiles[(b, dirn, step)]

            def chain(b, dirn, step, h):
                r = _RESTART[b]
                prev = cur_tile(b, dirn, step - 1)
                if dirn == "u":
                    gt, v = gus[b][h], r + step
                else:
                    gt, v = gds[b][h], r - step
                if h == 0:
                    w_tiles[(b, dirn, step)] = wp.tile(
                        [128, FREE], f16, tag="w", name=f"w{v}")
                wn = w_tiles[(b, dirn, step)]
                s = slice(h * H, (h + 1) * H)
                if eng_load["V"] + 326.0 <= eng_load["G"] + 427.0:
                    V.tensor_mul(wn[:, s], prev[:, s], gt[:, s])
                    eng_load["V"] += 326.0
                else:
                    G.tensor_mul(wn[:, s], prev[:, s], gt[:, s])
                    eng_load["G"] += 427.0
                emit_mm(wn, step, v, h)

            def run_seq(dirn, steps_of, seq):
                for item in seq:
                    if isinstance(item, tuple) and item[0] == "R":
                        _, blocks, h = item
                        for b in blocks:
                            emit_mm(w_r[b], 0, _RESTART[b], h)
                        continue
                    step, h = item
                    for b in range(N_BLOCKS):
                        if step in steps_of(b):
                            chain(b, dirn, step, h)

            def dsteps(b):
                return (1, 2, 3, 4) if _IS_LOW[b] else (1, 2, 3, 4, 5)

            def usteps(b):
                return (1, 2, 3, 4, 5) if _IS_LOW[b] else (1, 2, 3, 4)

            # low-block restarts sit in the even groups: open them now (h0;
            # h1 follows inside the d-phase stagger).  (4,1) runs before
            # (5,0) so the low even groups retire early; the high-restart
            # mms (odd groups) then slot in as those banks free.
            lows, highs = range(4), range(4, N_BLOCKS)
            for b in lows:
                emit_mm(w_r[b], 0, _RESTART[b], 0)
            run_seq("d", dsteps, [
                (1, 0), ("R", lows, 1), (2, 0), (1, 1), (3, 0), (2, 1),
                (4, 0), (3, 1), (4, 1), (5, 0), ("R", highs, 0),
                ("R", highs, 1), (5, 1)])
            run_seq("u", usteps, [
                (1, 0), (2, 0), (1, 1), (3, 0), (2, 1), (4, 0), (3, 1),
                (5, 0), (4, 1), (5, 1)])

    nc.finalize()
    return nc


def _host_inputs(inclination, sky_rot, line_broadening):
    f32 = np.float32
    f64 = np.float64
    inc = f32(inclination)
    rot = f32(sky_rot)
    lb = f32(line_broadening)
    ci, si = f32(np.cos(inc)), f32(np.sin(inc))
    cr, sr = f32(np.cos(rot)), f32(np.sin(rot))
    sig_sq = f32(lb * lb)

    lin = np.linspace(-CUBE_FOV, CUBE_FOV, IMAGE_RES, dtype=f32)
    z_labels = np.linspace(f32(VEL_MIN * M_TO_PC), f32(VEL_MAX * M_TO_PC),
                           VEL_RES, dtype=f32)
    dz = f64(z_labels[1]) - f64(z_labels[0])
    cv = -(z_labels.astype(f64) ** 2) / f64(sig_sq)

    sm = np.zeros((128, SM_COLS), dtype=f32)
    sm[:, _C_CIZ] = (ci * lin).astype(f32)
    sm[:, _C_Z2K] = (lin * lin).astype(f32)
    sm[:, _C_ISIG] = f32(1.0 / lb)
    sm[:, _C_TDS] = f32(2.0 * dz / f64(sig_sq))
    sm[:, _C_RC2] = f32(float(R_C) * float(R_C))
    for b in range(N_BLOCKS):
        r = _RESTART[b]
        if b < 4:
            sm[:, _C_ZV2 + b] = f32(z_labels[r] * f32(2.0 / sig_sq))
        sm[:, _C_CB + b] = f32(cv[r] + np.log(2.0) * PRESCALE)
        sm[:, _C_SU + b] = f32(np.exp(cv[r + 1] - cv[r]))
        sm[:, _C_SD + b] = f32(np.exp(cv[r - 1] - cv[r]))
    rat = dz * dz / f64(sig_sq)
    for n in range(2, 6):
        sm[:, _C_DN + n - 2] = f32(np.exp(-n * (n - 1) * rat))
    ones = np.ones((128, 64), dtype=np.float16)

    in_maps = []
    for c in range(N_CORES):
        x = lin[8 * c: 8 * c + 8][:, None]                 # [8,1]
        y = lin[None, :]                                   # [1,128]
        y1 = (sr * x + cr * y).astype(f32)
        rot_x = (cr * x - sr * y).astype(f32)
        pb = (si * y1).astype(f32).reshape(-1)
        pxy2 = (x * x + y * y).astype(f32).reshape(-1)
        pcv = (-si * V_MAX_PC * rot_x).astype(f32).reshape(-1)
        pkrow = np.concatenate([pb, pxy2, pcv]).astype(f32)  # [3*FREE]
        pk = np.ascontiguousarray(np.broadcast_to(pkrow, (128, 3 * FREE)))
        in_maps.append({"pk": pk, "sm": sm, "ones": ones})
    return in_maps


def _run(in_maps, trace=False, **kwargs):
    from concourse.bass_utils import run_bass_kernel_spmd
    if "nc" not in _CACHE:
        _CACHE["nc"] = _build_program()
    return run_bass_kernel_spmd(_CACHE["nc"], in_maps,
                                list(range(N_CORES)), trace=trace, **kwargs)


def _assemble(results, line_broadening):
    f32 = np.float32
    lb = f32(line_broadening)
    sig_sq = f32(lb * lb)
    pref = f32(1.0 / np.sqrt(2.0 * np.pi * sig_sq))
    scale = f32(pref / f32(VEL_UP * IMG_UP * IMG_UP) / f32(2.0 ** PRESCALE))
    parts = []
    for r in results:
        cube = np.asarray(r["out"]).reshape(16, 2, 4, 32, 4)  # vo,io,di,jo,dj
        pooled = cube.sum(axis=(2, 4), dtype=np.float32) * scale  # [16,2,32]
        parts.append(pooled.astype(f32))
    half = np.concatenate(parts, axis=1)
    full = np.empty((16, 32, 32), dtype=np.float32)
    full[:, :16, :] = half
    full[:, 16:, :] = half[::-1, ::-1, ::-1]
    return full


def kernel(inclination, sky_rot, line_broadening):
    in_maps = _host_inputs(inclination, sky_rot, line_broadening)
    res = _run(in_maps)
    return _assemble(res.results, line_broadening)
